# revision 29
# baseline (speedup 1.0000x reference)
"""Trainium2 Bass kernel for nn_CrossModalFusion.

Math: with seq_len=1 on both attention sides, softmax over the single key is
identically 1, so MHA collapses to  ctx = x_kv @ Wv.T @ Wo.T + (Wo @ bv + bo).
We fuse (Wv.T @ Wo.T) into one [d, d] weight on the host, so each modality is a
single [B,d]x[d,d] matmul, a residual add, a LayerNorm, plus the final
concat([img_out, txt_out, img_out*txt_out]).

Sharding: pure data parallel over the batch dim across 8 NeuronCores, weights
replicated, no collectives.

Device data is fp16 (full PE rate like bf16, but ~2^-11 rounding). All input
tensors are host-packed into SBUF-ready [128, big] slabs (slab row p holds
exactly partition p's bytes) so every DMA load is a plain 2D slice with large
contiguous descriptors. Per-core layout (Bs = 4096 rows):
  - img_n/txt_n  packed naturals (residual input)
  - imgT/txtT    packed transposes (matmul stationary operand: the PE wants
                 the contraction dim on partitions)
  - w_it/w_ti    packed fused weights [d_in, d_out]
  - out          [Bs, 3d] f16 (one merged store per b-tile; host casts to f32 —
                 fp16 rounding ~5e-4 rel, well under the 2e-2 gate, and it
                 halves the dominant store traffic: 88->63 MB/core total)
All DMA on the SP HWDGE ring (engine queues are strict FIFO on HW: DMA
triggers must not share a queue with compute another tile depends on).
"""

import os
import numpy as np
import ml_dtypes

B, D, NCORES = 32768, 1024, 8
BS = B // NCORES          # 4096 rows per core
PT = 128                  # partition tile (rows per b-tile)
NBT = BS // PT            # 32 b-tiles per core
KT = D // PT              # 8 k-tiles of the contraction
NH = 512                  # psum half width (one fp32 PSUM bank)
CHUNK = 512               # b-columns per transposed-input chunk load
NCHUNK = BS // CHUNK
NATCH = 512               # rows per natural-input chunk load
EPS = 1e-5
XDT = np.float16  # device dtype for x / weights (fp16: full PE rate, ~2^-11 rounding)

# Which program kernel() ships (and test.py times as "the" kernel).
# "full8" = matmul stationary operand (xT) in fp8-e3m4: FWL loads weights 4
# per 32-bit read instead of 2, halving the exposed LDWEIGHTS cost; W (the
# moving operand) stays fp16, so only x is quantized (~2^-5 rounding).
DEFAULT_VARIANT = "full8"

_CACHE = {}


def _build_program(repeats, has_bias, has_affine, variant="full"):
    """variant: "<base>[:<load_eng>:<store_eng>]"
    base: full | dmaonly | peonly (loads+matmuls+residual only) |
    nomm (full minus matmuls — reads stale psum) |
    mmonly (xt loads + matmuls, nothing else) |
    mmshared (mmonly with one fixed lhsT — probes LDWEIGHTS dedup) |
    mmldw (mmonly with explicit ldweights per k — probes LDW pairing)"""
    import concourse.bass as bass
    import concourse.tile as tile
    from concourse import bacc, mybir

    # default: loads trigger on SP (SyncE has nothing else to do), stores on
    # GpSimd (SWDGE) so a store's wait-for-compute never queues ahead of the
    # next tile's loads on the same engine FIFO; nat loads optionally on a
    # separate ring (4th field) so they never queue behind big chunk loads
    parts = (variant.split(":") + ["sp", "gp", ""])[:4]
    base, load_eng_name, store_eng_name, nat_eng_name = parts
    nat_eng_name = nat_eng_name or "act"

    f32 = mybir.dt.float32
    f16 = mybir.dt.float16
    f8 = mybir.dt.float8e3
    AF = mybir.ActivationFunctionType
    ALU = mybir.AluOpType

    nc = bacc.Bacc("TRN2", enable_partition_id=False)

    # packed layouts: loads are [128, big] row-contiguous slabs (row p of the
    # slab is exactly partition p's bytes)
    NATB = NATCH // PT
    img_n = nc.declare_dram_parameter(
        "img_n", [(BS // NATCH) * PT, NATB * D], f16, isOutput=False)
    txt_n = nc.declare_dram_parameter(
        "txt_n", [(BS // NATCH) * PT, NATB * D], f16, isOutput=False)
    imgT = nc.declare_dram_parameter(
        "imgT", [NCHUNK * PT, KT * CHUNK], f16, isOutput=False)
    txtT = nc.declare_dram_parameter(
        "txtT", [NCHUNK * PT, KT * CHUNK], f16, isOutput=False)
    w_it = nc.declare_dram_parameter("w_it", [PT, KT * D], f16, isOutput=False)
    w_ti = nc.declare_dram_parameter("w_ti", [PT, KT * D], f16, isOutput=False)
    imgT8 = txtT8 = None
    if base in ("mmfp8l", "full8"):
        imgT8 = nc.declare_dram_parameter(
            "imgT8", [NCHUNK * PT, KT * CHUNK], f8, isOutput=False)
        txtT8 = nc.declare_dram_parameter(
            "txtT8", [NCHUNK * PT, KT * CHUNK], f8, isOutput=False)
    bias_d = affine_d = None
    if has_bias:
        bias_d = nc.declare_dram_parameter("bias", [2, D], f32, isOutput=False)
    if has_affine:
        affine_d = nc.declare_dram_parameter("affine", [4, D], f32, isOutput=False)
    out_d = nc.declare_dram_parameter("out", [BS, 3 * D], f16, isOutput=True)

    with tile.TileContext(nc) as tc:
        _engs = {"sp": nc.sync, "act": nc.scalar, "gp": nc.gpsimd}
        load_e = _engs[load_eng_name]
        store_e = _engs[store_eng_name]
        nat_e = _engs[nat_eng_name]
        with (
            tc.tile_pool(name="singles", bufs=1) as singles,
            tc.tile_pool(name="wpool", bufs=1) as wpool,
            tc.tile_pool(name="xtpool", bufs=3) as xtpool,
            tc.tile_pool(name="natpool", bufs=3) as natpool,
            tc.tile_pool(name="ypool", bufs=3) as ypool,
            tc.tile_pool(name="outpool", bufs=3) as outpool,
            tc.tile_pool(name="smalls", bufs=6) as smalls,
            tc.tile_pool(name="psum", bufs=2, space=bass.MemorySpace.PSUM) as psum,
        ):
            def body():
                eps_t = singles.tile([PT, 1], f32, tag="eps")
                nc.vector.memset(eps_t, EPS)

                w_sb = {}
                for mod, w_d in (("it", w_it), ("ti", w_ti)):
                    w = wpool.tile([PT, KT, D], f16, tag=f"w_{mod}", name=f"w_{mod}")
                    load_e.dma_start(
                        out=w, in_=w_d.rearrange("p (k n) -> p k n", k=KT))
                    w_sb[mod] = w

                bias_bc, aff_bc = {}, {}
                if has_bias:
                    for i, mod in enumerate(("it", "ti")):
                        t = singles.tile([PT, D], f32, tag=f"bias_{mod}",
                                         name=f"bias_{mod}")
                        s = bias_d[i : i + 1, :]
                        s = bass.AP(tensor=s.tensor, offset=s.offset,
                                    ap=[[0, PT], [1, D]])
                        load_e.dma_start(out=t, in_=s)
                        bias_bc[mod] = t
                if has_affine:
                    for i, nm in enumerate(("g_img", "b_img", "g_txt", "b_txt")):
                        t = singles.tile([PT, D], f32, tag=f"aff_{nm}", name=nm)
                        s = affine_d[i : i + 1, :]
                        s = bass.AP(tensor=s.tensor, offset=s.offset,
                                    ap=[[0, PT], [1, D]])
                        load_e.dma_start(out=t, in_=s)
                        aff_bc[nm] = t

                MODS = (("it", ("g_img", "b_img")), ("ti", ("g_txt", "b_txt")))

                def phase1(xt_sb, bb, g, bi):
                    """Matmuls + residual add (+row sums) for one b-tile."""
                    ysum = smalls.tile([PT, 2], f32, tag="ysum", name="ysum")
                    sumsq = None
                    if base not in ("peonly", "sttnonat", "sttcopy", "sttact"):
                        sumsq = smalls.tile([PT, 2], f32, tag="sumsq",
                                            name="sumsq")
                    ys = {}
                    for mi, (mod, gb) in enumerate(MODS):
                        nat = None
                        if base in ("sttnonat", "sttcopy", "sttact"):
                            # probe: skip the nat DMA; read resident xt data
                            # as the second STT operand instead
                            if base == "sttnonat":
                                nat = xt_sb[mod][:, 0:2, 0:NH]
                        else:
                            x_nat_d = img_n if mod == "it" else txt_n
                            nat = natpool.tile([PT, D], f16, tag=f"nat_{mod}",
                                               name=f"nat_{mod}")
                            nat_e.dma_start(
                                out=nat,
                                in_=x_nat_d[g * PT : (g + 1) * PT,
                                            bi * D : (bi + 1) * D])

                        ps = psum.tile([PT, D], f32, tag=f"ps_{mod}",
                                       name=f"ps_{mod}")
                        xt = xt_sb[mod]
                        if base != "nomm":
                            for k in range(KT):
                                lhsT = xt[:, k, bb * PT : (bb + 1) * PT]
                                for h in range(2):
                                    nc.tensor.matmul(
                                        ps[:, h * NH : (h + 1) * NH],
                                        lhsT,
                                        w_sb[mod][:, k, h * NH : (h + 1) * NH],
                                        start=(k == 0),
                                        stop=(k == KT - 1))

                        y = ypool.tile([PT, D], f32, tag=f"y_{mod}",
                                       name=f"y_{mod}")
                        if base == "sttcopy":
                            # probe: plain psum->sbuf copy, no 2nd operand
                            nc.vector.tensor_copy(y, ps)
                            ys[mod] = y
                            continue
                        if base == "sttact":
                            # probe: psum->sbuf drain on the scalar engine
                            nc.scalar.activation(y, ps, func=AF.Identity)
                            ys[mod] = y
                            continue
                        # y = ps + nat, and row-sum(y) in the same DVE pass
                        # (tensor_tensor_reduce hard-faults trn2 hardware;
                        # scalar_tensor_tensor with accum_out is equivalent)
                        nc.vector.scalar_tensor_tensor(
                            y, ps, 1.0, nat, op0=ALU.mult, op1=ALU.add,
                            accum_out=ysum[:, mi : mi + 1])
                        if has_bias:
                            # redo the sum after the bias add
                            nc.vector.scalar_tensor_tensor(
                                y, y, 1.0, bias_bc[mod],
                                op0=ALU.mult, op1=ALU.add,
                                accum_out=ysum[:, mi : mi + 1])
                        if base not in ("peonly", "sttnonat", "sttcopy", "sttact"):
                            # row-sum(y^2) on the (otherwise idle) scalar eng
                            ysq = ypool.tile([PT, D], f16, tag="ysq",
                                             name="ysq")
                            nc.scalar.activation(
                                ysq, y, func=AF.Square,
                                accum_out=sumsq[:, mi : mi + 1])
                        ys[mod] = y
                    return ysum, sumsq, ys

                def phase2(st):
                    """LN stats chain, normalize-apply, product, store —
                    emitted one b-tile late (software pipeline) so neither
                    FIFO engine stalls the next tile's front-end work."""
                    ysum, sumsq, ys, rows = st
                    negmu = smalls.tile([PT, 2], f32, tag="negmu", name="negmu")
                    nc.vector.tensor_scalar_mul(negmu, ysum, -1.0 / D)
                    musq = smalls.tile([PT, 2], f32, tag="musq", name="musq")
                    nc.vector.tensor_mul(musq, negmu, negmu)
                    var = smalls.tile([PT, 2], f32, tag="var", name="var")
                    nc.vector.scalar_tensor_tensor(
                        var, sumsq, 1.0 / D, musq,
                        op0=ALU.mult, op1=ALU.subtract)
                    std = smalls.tile([PT, 2], f32, tag="std", name="std")
                    nc.scalar.activation(
                        std, var, func=AF.Sqrt, bias=eps_t, scale=1.0)
                    rstd = smalls.tile([PT, 2], f32, tag="rstd", name="rstd")
                    nc.vector.reciprocal(rstd, std)
                    nmr = smalls.tile([PT, 2], f32, tag="nmr", name="nmr")
                    nc.vector.tensor_mul(nmr, negmu, rstd)

                    out_t = outpool.tile([PT, 3, D], f16, tag="out",
                                         name="out_t")
                    for mi, (mod, gb) in enumerate(MODS):
                        ln = out_t[:, mi, :]
                        # (y - mu) * rstd == y*rstd + (-mu*rstd), f16 out
                        nc.scalar.activation(
                            ln, ys[mod], func=AF.Identity,
                            bias=nmr[:, mi : mi + 1],
                            scale=rstd[:, mi : mi + 1])
                        if has_affine:
                            nc.vector.tensor_mul(ln, ln, aff_bc[gb[0]])
                            nc.vector.tensor_add(ln, ln, aff_bc[gb[1]])

                    nc.vector.tensor_mul(
                        out_t[:, 2, :], out_t[:, 0, :], out_t[:, 1, :])
                    store_e.dma_start(
                        out=out_d[rows, :].rearrange("p (s n) -> p s n", s=3),
                        in_=out_t)

                def phase1_mm(xt_sb, bb, mode):
                    """Matmul-stream-only probes (psum written, never read)."""
                    for mod, _gb in MODS:
                        ps = psum.tile([PT, D], f32, tag=f"ps_{mod}",
                                       name=f"ps_{mod}")
                        xt = xt_sb[mod]
                        if mode == "mmkmajor":
                            # h outer, k inner: runs of 8 same-bank MMs,
                            # LDW:MM = 1:1 (each lhsT loaded twice overall)
                            for h in range(2):
                                for k in range(KT):
                                    lhsT = xt[:, k, bb * PT : (bb + 1) * PT]
                                    nc.tensor.matmul(
                                        ps[:, h * NH : (h + 1) * NH],
                                        lhsT,
                                        w_sb[mod][:, k, h * NH : (h + 1) * NH],
                                        start=(k == 0),
                                        stop=(k == KT - 1))
                            continue
                        if mode == "mm1bank":
                            # half the work: single bank, LDW:MM = 1:1 —
                            # bsp_matmul-shaped stream (perf probe only)
                            for k in range(KT):
                                lhsT = xt[:, k, bb * PT : (bb + 1) * PT]
                                nc.tensor.matmul(
                                    ps[:, 0:NH],
                                    lhsT,
                                    w_sb[mod][:, k, 0:NH],
                                    start=(k == 0),
                                    stop=(k == KT - 1))
                            continue
                        for k in range(KT):
                            if mode == "mmshared":
                                lhsT = xt[:, 0, 0:PT]
                            else:
                                lhsT = xt[:, k, bb * PT : (bb + 1) * PT]
                            if mode == "mmldw":
                                nc.tensor.ldweights(lhsT)
                            for h in range(2):
                                nc.tensor.matmul(
                                    ps[:, h * NH : (h + 1) * NH],
                                    lhsT,
                                    w_sb[mod][:, k, h * NH : (h + 1) * NH],
                                    start=(k == 0),
                                    stop=(k == KT - 1))

                pending = None
                if base in ("mmfp8l", "full8"):
                    xt_srcs, xt_dt = (("it", txtT8), ("ti", imgT8)), f8
                else:
                    xt_srcs, xt_dt = (("it", txtT), ("ti", imgT)), f16
                for c in range(NCHUNK):
                    xt_sb = {}
                    for mod, xT_d in xt_srcs:
                        # "it" produces img_ctx from txt; "ti" the reverse
                        xt = xtpool.tile([PT, KT, CHUNK], xt_dt,
                                         tag=f"xt_{mod}", name=f"xt_{mod}")
                        load_e.dma_start(
                            out=xt,
                            in_=xT_d[c * PT : (c + 1) * PT, :].rearrange(
                                "p (k b) -> p k b", k=KT))
                        xt_sb[mod] = xt

                    for bb in range(CHUNK // PT):
                        b0 = c * CHUNK + bb * PT
                        rows = slice(b0, b0 + PT)
                        g, bi = b0 // NATCH, (b0 % NATCH) // PT

                        if base in ("mmonly", "mmshared", "mmldw",
                                    "mmkmajor", "mm1bank", "mmfp8l"):
                            phase1_mm(xt_sb, bb, base)
                            continue
                        if base == "dmaonly":
                            # store loaded data so nothing is elidable
                            store_e.dma_start(
                                out=out_d[rows, :].rearrange(
                                    "p (s n) -> p s n", s=6),
                                in_=xt_sb["it"][:, 0:6, :])
                            continue

                        ysum, sumsq, ys = phase1(xt_sb, bb, g, bi)
                        if base in ("peonly", "sttnonat", "sttcopy", "sttact"):
                            continue
                        if pending is not None:
                            phase2(pending)
                        pending = (ysum, sumsq, ys, rows)
                if pending is not None:
                    phase2(pending)

            if repeats == 1:
                body()
            else:
                with tc.For_i(0, repeats, 1):
                    body()

    nc.finalize()
    return nc


def _get_exec(repeats=1, has_bias=False, has_affine=False, variant="full"):
    key = (repeats, has_bias, has_affine, variant)
    if key in _CACHE:
        return _CACHE[key]

    import jax
    from jax.experimental.shard_map import shard_map
    from jax.sharding import Mesh, PartitionSpec
    from concourse import mybir
    from concourse.bass2jax import (
        _bass_exec_p,
        install_neuronx_cc_hook,
        partition_id_tensor,
    )

    install_neuronx_cc_hook()
    nc = _build_program(repeats, has_bias, has_affine, variant)

    partition_name = nc.partition_id_tensor.name if nc.partition_id_tensor else None
    in_names, out_names, out_avals = [], [], []
    for alloc in nc.m.functions[0].allocations:
        if not isinstance(alloc, mybir.MemoryLocationSet):
            continue
        name = alloc.memorylocations[0].name
        if alloc.kind == "ExternalInput":
            if name != partition_name:
                in_names.append(name)
        elif alloc.kind == "ExternalOutput":
            out_names.append(name)
            out_avals.append(
                jax.core.ShapedArray(tuple(alloc.tensor_shape), mybir.dt.np(alloc.dtype))
            )
    n_params = len(in_names)
    all_in_names = list(in_names) + out_names
    if partition_name is not None:
        all_in_names.append(partition_name)
    all_in_names = tuple(all_in_names)

    def _body(*args):
        operands = list(args)
        if partition_name is not None:
            operands.append(partition_id_tensor())
        return tuple(
            _bass_exec_p.bind(
                *operands,
                out_avals=tuple(out_avals),
                in_names=all_in_names,
                out_names=tuple(out_names),
                lowering_input_output_aliases=(),
                sim_require_finite=True,
                sim_require_nnan=True,
                nc=nc,
            )
        )

    devices = jax.devices()[:NCORES]
    assert len(devices) == NCORES, f"need {NCORES} devices, got {len(devices)}"
    mesh = Mesh(np.asarray(devices), ("core",))
    nspecs = n_params + len(out_names)
    fn = jax.jit(
        shard_map(
            _body,
            mesh=mesh,
            in_specs=(PartitionSpec("core"),) * nspecs,
            out_specs=(PartitionSpec("core"),) * len(out_names),
            check_rep=False,
        ),
        keep_unused=True,
    )
    entry = (fn, in_names, out_names, out_avals, mesh)
    _CACHE[key] = entry
    return entry


def _prep_inputs(inputs):
    """Host-side prep: fuse weights, cast, transpose. Returns (global input
    arrays dict keyed by dram param name, has_bias, has_affine)."""
    img = np.asarray(inputs["img"], np.float32)
    txt = np.asarray(inputs["txt"], np.float32)

    glob = {}
    has_bias = False
    bias_rows = []
    w_glob = {}
    for mod, wi, bi, wo, bo in (
        ("it", "Wi_it", "bi_it", "Wo_it", "bo_it"),
        ("ti", "Wi_ti", "bi_ti", "Wo_ti", "bo_ti"),
    ):
        Wi = np.asarray(inputs[wi], np.float32)
        Wo = np.asarray(inputs[wo], np.float32)
        bi = np.asarray(inputs[bi], np.float32)
        bo = np.asarray(inputs[bo], np.float32)
        Wv = Wi[2 * D : 3 * D]               # v = x_kv @ Wv.T + bv
        Wf = (Wv.T @ Wo.T).astype(XDT)      # ctx = x_kv @ Wf, [d_in, d_out]
        bf = Wo @ bi[2 * D : 3 * D] + bo
        w_glob[mod] = Wf
        bias_rows.append(bf)
        if np.any(bf != 0.0):
            has_bias = True

    aff = [np.asarray(inputs[k], np.float32)
           for k in ("g_img", "b_img", "g_txt", "b_txt")]
    has_affine = bool(
        np.any(aff[0] != 1.0) or np.any(aff[1] != 0.0)
        or np.any(aff[2] != 1.0) or np.any(aff[3] != 0.0)
    )

    img16 = img.astype(XDT)
    txt16 = txt.astype(XDT)
    NATB = NATCH // PT
    NG = BS // NATCH  # natural-chunk groups per core

    def pack_nat(x16):
        # [NCORES*BS, D] -> per-core [(NG*PT), NATB*D] slabs, concatenated:
        # row (g,p) holds rows g*NATCH + bb*PT + p for bb in range(NATB)
        r = x16.reshape(NCORES, NG, NATB, PT, D).transpose(0, 1, 3, 2, 4)
        return np.ascontiguousarray(r).reshape(NCORES * NG * PT, NATB * D)

    def pack_xT(x16):
        # per-core transposed [(NCHUNK*PT), KT*CHUNK]: row (c,p) holds
        # feature rows k*PT+p over batch-columns of chunk c
        xt = x16.reshape(NCORES, BS, D).transpose(0, 2, 1)  # [NC, D, BS]
        r = xt.reshape(NCORES, KT, PT, NCHUNK, CHUNK).transpose(0, 3, 2, 1, 4)
        return np.ascontiguousarray(r).reshape(NCORES * NCHUNK * PT, KT * CHUNK)

    def pack_w(w):
        r = w.reshape(KT, PT, D).transpose(1, 0, 2)
        return np.ascontiguousarray(r).reshape(PT, KT * D)

    glob["img_n"] = pack_nat(img16)
    glob["txt_n"] = pack_nat(txt16)
    glob["imgT"] = pack_xT(img16)
    glob["txtT"] = pack_xT(txt16)
    glob["imgT8"] = glob["imgT"].astype(ml_dtypes.float8_e3m4)
    glob["txtT8"] = glob["txtT"].astype(ml_dtypes.float8_e3m4)
    wpk_it = pack_w(w_glob["it"])
    wpk_ti = pack_w(w_glob["ti"])
    glob["w_it"] = np.broadcast_to(wpk_it, (NCORES, PT, KT * D)).reshape(NCORES * PT, KT * D).copy()
    glob["w_ti"] = np.broadcast_to(wpk_ti, (NCORES, PT, KT * D)).reshape(NCORES * PT, KT * D).copy()
    if has_bias:
        b = np.stack(bias_rows).astype(np.float32)  # [2, D]
        glob["bias"] = np.broadcast_to(b, (NCORES, 2, D)).reshape(NCORES * 2, D).copy()
    if has_affine:
        a = np.stack(aff).astype(np.float32)  # [4, D]
        glob["affine"] = np.broadcast_to(a, (NCORES, 4, D)).reshape(NCORES * 4, D).copy()
    return glob, has_bias, has_affine


def kernel(**inputs):
    glob, has_bias, has_affine = _prep_inputs(inputs)
    fn, in_names, out_names, out_avals, mesh = _get_exec(
        1, has_bias, has_affine, DEFAULT_VARIANT)
    args = [glob[n] for n in in_names]
    zeros = [
        np.zeros((NCORES * av.shape[0], *av.shape[1:]), av.dtype) for av in out_avals
    ]
    outs = fn(*args, *zeros)
    return np.asarray(outs[0]).astype(np.float32)


if __name__ == "__main__":
    rng = np.random.default_rng(0)
    fake = {
        "img": rng.standard_normal((B, D), np.float32),
        "txt": rng.standard_normal((B, D), np.float32),
        "Wi_it": rng.standard_normal((3 * D, D), np.float32) / 32,
        "bi_it": np.zeros(3 * D, np.float32),
        "Wo_it": rng.standard_normal((D, D), np.float32) / 32,
        "bo_it": np.zeros(D, np.float32),
        "Wi_ti": rng.standard_normal((3 * D, D), np.float32) / 32,
        "bi_ti": np.zeros(3 * D, np.float32),
        "Wo_ti": rng.standard_normal((D, D), np.float32) / 32,
        "bo_ti": np.zeros(D, np.float32),
        "g_img": np.ones(D, np.float32),
        "b_img": np.zeros(D, np.float32),
        "g_txt": np.ones(D, np.float32),
        "b_txt": np.zeros(D, np.float32),
    }
    out = kernel(**fake)
    print(out.shape, out.dtype)

